# revision 11
# baseline (speedup 1.0000x reference)
"""Trainium2 Bass kernel for causal multi-head attention block (nn_Attention).

Reference computation (B=4, S=2048, EMB=1024, H=16, Dh=64):
    qkv = x @ w_qkv + b_qkv ; q,k,v = split(qkv)
    out = softmax(causal(q k^T / sqrt(Dh))) @ v   (per head)
    y = merge_heads(out) @ w_fc + b_fc

Sharding: 8 cores = 4 batches x 2 interleaved query-stripe halves.
Core (b, half) owns query stripes {128*(2i+half) : i=0..7} of batch b.
Each core computes K/V for the full sequence of its batch (zero-collective
SPMD), Q only for its own 1024 queries, causally-tapered attention, and the
final FC for its own output rows. Causal boundary differences between the
two halves are data (mask inputs), not control flow, so all 8 cores run one
graph.

On-device layouts (host pre-transposes x):
    xT_kv [1024,2048] = x[b].T          xT_q [1024,1024] = x[b, stripes].T
    K^T, Q^T computed weight-stationary (lhsT=W chunk, rhs=xT)
    V computed activation-stationary (lhsT=xT chunk, rhs=Wv) -> [s, d] layout
      with a ones-column per head (softmax denominators fall out of PV matmul)
    scores^T [kv,q] = lhsT(K^T).T @ rhs(Q^T); exp on ScalarE (scale=1/8)
    PV: num^T [65, q] = lhsT(V_aug).T @ rhs(probs^T), accumulated over kv tiles
    FC: out [q, c] = lhsT(attn^T).T @ rhs(W_fc)  -> direct row-major output
"""

import numpy as np
import ml_dtypes

B = 4
S = 2048
EMB = 1024
HEADS = 16
DH = 64
NCORES = 8
NSTRIPE = 8          # query stripes per core
QW = NSTRIPE * 128   # queries per core
KV_TILES = 16        # 2048 / 128

BF16 = ml_dtypes.bfloat16

_compiled = None


def _build():
    from concourse import bacc, tile, mybir

    nc = bacc.Bacc("TRN2", target_bir_lowering=False, debug=False,
                   num_devices=NCORES)
    f32 = mybir.dt.float32
    bf16 = mybir.dt.bfloat16
    Exp = mybir.ActivationFunctionType.Exp
    Copy = mybir.ActivationFunctionType.Copy
    Mult = mybir.AluOpType.mult
    Add = mybir.AluOpType.add

    xT_kv = nc.dram_tensor("xT_kv", [EMB, S], bf16, kind="ExternalInput")
    xT_q = nc.dram_tensor("xT_q", [EMB, QW], bf16, kind="ExternalInput")
    wqkv = nc.dram_tensor("wqkv", [EMB, 3 * EMB], bf16, kind="ExternalInput")
    wfc = nc.dram_tensor("wfc", [EMB, EMB], bf16, kind="ExternalInput")
    bqkv_t = nc.dram_tensor("bqkv_t", [128, 24], f32, kind="ExternalInput")
    bv_row = nc.dram_tensor("bv_row", [1, EMB], f32, kind="ExternalInput")
    bfc_row = nc.dram_tensor("bfc_row", [1, EMB], f32, kind="ExternalInput")
    masks = nc.dram_tensor("masks", [128, 256], bf16, kind="ExternalInput")
    out = nc.dram_tensor("out", [QW, EMB], f32, kind="ExternalOutput")

    with tile.TileContext(nc) as tc:
        with (
            tc.tile_pool(name="consts", bufs=1) as consts,
            tc.tile_pool(name="vpool", bufs=1) as vpool,
            tc.tile_pool(name="ktpool", bufs=1) as ktpool,
            tc.tile_pool(name="qtpool", bufs=1) as qtpool,
            tc.tile_pool(name="attn", bufs=1) as attnp,
            tc.tile_pool(name="pwork", bufs=4, space="PSUM") as pwork,
            tc.tile_pool(name="pnum", bufs=2, space="PSUM") as pnum,
        ):
            # ---- constants ----
            mask_sb = consts.tile([128, 256], bf16, tag="mask")
            nc.sync.dma_start(out=mask_sb[:], in_=masks.ap()[:])
            bqkv_sb = consts.tile([128, 24], f32, tag="bqkv")
            nc.sync.dma_start(out=bqkv_sb[:], in_=bqkv_t.ap()[:])
            bv_bc = consts.tile([128, EMB], f32, tag="bvbc")
            bfc_bc = consts.tile([128, EMB], f32, tag="bfcbc")
            with tc.tile_pool(name="brows", bufs=1) as brows:
                bv_sb = brows.tile([1, EMB], f32, tag="bvrow")
                nc.sync.dma_start(out=bv_sb[:], in_=bv_row.ap()[:])
                bfc_sb = brows.tile([1, EMB], f32, tag="bfcrow")
                nc.sync.dma_start(out=bfc_sb[:], in_=bfc_row.ap()[:])
                nc.gpsimd.partition_broadcast(bv_bc[:], bv_sb[:], channels=128)
                nc.gpsimd.partition_broadcast(bfc_bc[:], bfc_sb[:], channels=128)

            # persistent activation tiles
            v_sb = [vpool.tile([128, HEADS, DH + 8], bf16, tag=f"v{st}", name=f"v{st}")
                    for st in range(KV_TILES)]
            kt_sb = [ktpool.tile([128, S], bf16, tag=f"kt{p}", name=f"kt{p}")
                     for p in range(NSTRIPE)]
            qt_sb = [qtpool.tile([128, QW], bf16, tag=f"qt{p}", name=f"qt{p}")
                     for p in range(NSTRIPE)]
            attn_sb = [attnp.tile([128, QW], bf16, tag=f"at{p}", name=f"at{p}")
                       for p in range(NSTRIPE)]

            # ================= phase 1: projections =================
            with (
                tc.tile_pool(name="xkv", bufs=1) as xkvp,
                tc.tile_pool(name="xq", bufs=1) as xqp,
                tc.tile_pool(name="wq", bufs=2) as wqp,
            ):
                xkv = [xkvp.tile([128, S], bf16, tag=f"xk{e}", name=f"xk{e}")
                       for e in range(8)]
                xq = [xqp.tile([128, QW], bf16, tag=f"xq{e}", name=f"xq{e}")
                      for e in range(8)]
                for e in range(8):
                    nc.sync.dma_start(out=xkv[e][:],
                                      in_=xT_kv.ap()[128 * e:128 * e + 128, :])
                    nc.sync.dma_start(out=xq[e][:],
                                      in_=xT_q.ap()[128 * e:128 * e + 128, :])

                # V projection: out[s, d] ; lhsT = xT chunk, rhs = Wv cols
                wv = [wqp.tile([128, EMB], bf16, tag=f"ws{e}", name=f"wv{e}")
                      for e in range(8)]
                for e in range(8):
                    nc.sync.dma_start(
                        out=wv[e][:],
                        in_=wqkv.ap()[128 * e:128 * e + 128, 2048:3072])
                for st in range(KV_TILES):
                    nc.vector.memset(v_sb[st][:, :, DH:DH + 1], 1.0)
                    for dc in range(2):
                        ps = pwork.tile([128, 512], f32, tag="pw", name="pw")
                        for e in range(8):
                            nc.tensor.matmul(
                                ps[:],
                                lhsT=xkv[e][:, 128 * st:128 * st + 128],
                                rhs=wv[e][:, 512 * dc:512 * dc + 512],
                                start=(e == 0), stop=(e == 7),
                            )
                        nc.vector.tensor_tensor(
                            out=v_sb[st][:, 8 * dc:8 * dc + 8, 0:DH],
                            in0=ps[:],
                            in1=bv_bc[:, 512 * dc:512 * dc + 512],
                            op=Add,
                        )

                # K^T projection (pair-tiles of 128 cols = 2 heads)
                wk = [wqp.tile([128, EMB], bf16, tag=f"ws{e}", name=f"wk{e}")
                      for e in range(8)]
                for e in range(8):
                    nc.sync.dma_start(
                        out=wk[e][:],
                        in_=wqkv.ap()[128 * e:128 * e + 128, 1024:2048])
                for p in range(8):
                    for sc in range(4):
                        ps = pwork.tile([128, 512], f32, tag="pw", name="pw")
                        for e in range(8):
                            nc.tensor.matmul(
                                ps[:],
                                lhsT=wk[e][:, 128 * p:128 * p + 128],
                                rhs=xkv[e][:, 512 * sc:512 * sc + 512],
                                start=(e == 0), stop=(e == 7),
                            )
                        nc.vector.tensor_scalar_add(
                            kt_sb[p][:, 512 * sc:512 * sc + 512], ps[:],
                            bqkv_sb[:, 8 + p:9 + p],
                        )

                # Q^T projection (core's own queries only)
                wqt = [wqp.tile([128, EMB], bf16, tag=f"ws{e}", name=f"wqt{e}")
                       for e in range(8)]
                for e in range(8):
                    nc.sync.dma_start(
                        out=wqt[e][:],
                        in_=wqkv.ap()[128 * e:128 * e + 128, 0:1024])
                for p in range(8):
                    for qc in range(2):
                        ps = pwork.tile([128, 512], f32, tag="pw", name="pw")
                        for e in range(8):
                            nc.tensor.matmul(
                                ps[:],
                                lhsT=wqt[e][:, 128 * p:128 * p + 128],
                                rhs=xq[e][:, 512 * qc:512 * qc + 512],
                                start=(e == 0), stop=(e == 7),
                            )
                        nc.vector.tensor_scalar_add(
                            qt_sb[p][:, 512 * qc:512 * qc + 512], ps[:],
                            bqkv_sb[:, p:p + 1],
                        )

            # ================= phase 2: attention =================
            with (
                tc.tile_pool(name="probs", bufs=6) as probsp,
                tc.tile_pool(name="numsb", bufs=3) as numsbp,
                tc.tile_pool(name="rbp", bufs=3) as rbp,
                tc.tile_pool(name="wfc", bufs=1) as wfcp,
            ):
                wfc_sb = [wfcp.tile([128, EMB], bf16, tag=f"wf{e}", name=f"wf{e}")
                          for e in range(8)]
                for e in range(8):
                    nc.sync.dma_start(out=wfc_sb[e][:],
                                      in_=wfc.ap()[128 * e:128 * e + 128, :])
                for p in range(8):          # head pair p -> heads 2p, 2p+1
                    num_ps = [pnum.tile([DH + 1, QW], f32, tag="pn", name="pn")
                              for _ in range(2)]
                    for k in range(KV_TILES):
                        q0 = 128 * (k // 2)
                        # absolute q-chunk boundaries, split at PSUM bank edge
                        bounds = [q0, 512, QW] if q0 < 512 else [q0, QW]
                        for hh in range(2):
                            h = 2 * p + hh
                            lo, hi = 64 * hh, 64 * hh + 64
                            probs = probsp.tile([128, QW], bf16, tag="pr", name="pr")
                            for ci in range(len(bounds) - 1):
                                a, bnd = bounds[ci], bounds[ci + 1]
                                n = bnd - a
                                ps = pwork.tile([128, 512], f32, tag="pw", name="pw")
                                nc.tensor.matmul(
                                    ps[:, 0:n],
                                    lhsT=kt_sb[p][lo:hi, 128 * k:128 * k + 128],
                                    rhs=qt_sb[p][lo:hi, a:bnd],
                                    start=True, stop=True,
                                )
                                nc.scalar.activation(
                                    probs[:, a:bnd], ps[:, 0:n], Exp,
                                    scale=0.125,
                                )
                            # mask first 128 q-cols (diag/invalid stripe)
                            nc.vector.tensor_tensor(
                                out=probs[:, q0:q0 + 128],
                                in0=probs[:, q0:q0 + 128],
                                in1=mask_sb[:, 128 * (k % 2):128 * (k % 2) + 128],
                                op=Mult,
                            )
                            # PV accumulate
                            for ci in range(len(bounds) - 1):
                                a, bnd = bounds[ci], bounds[ci + 1]
                                stop = (k == 7 and bnd == 512) or \
                                       (k == 15 and bnd == QW)
                                nc.tensor.matmul(
                                    num_ps[hh][:, a:bnd],
                                    lhsT=v_sb[k][:, h, 0:DH + 1],
                                    rhs=probs[:, a:bnd],
                                    start=(k == 0), stop=stop,
                                )
                    # normalize: attn^T = num^T[0:64] * (1/num^T[64])
                    for hh in range(2):
                        # fused copy+recip of the sums row (PSUM p64 -> SBUF
                        # p64), then down to p0 (partition_broadcast only
                        # reads partition 0 on HW), broadcast, multiply.
                        rr = numsbp.tile([DH + 1, QW], f32, tag="ns", name="ns")
                        nc.vector.tensor_copy(
                            rr[DH:DH + 1, :], num_ps[hh][DH:DH + 1, :])
                        r0 = rbp.tile([1, QW], f32, tag="r0", name="r0")
                        nc.sync.dma_start(out=r0[:], in_=rr[DH:DH + 1, :])
                        nc.vector.reciprocal_approx_fast(out=r0[:], in_=r0[:])
                        rb = rbp.tile([DH, QW], f32, tag="rb", name="rb")
                        nc.gpsimd.partition_broadcast(
                            rb[:], r0[:], channels=DH)
                        nc.vector.tensor_tensor(
                            out=attn_sb[p][64 * hh:64 * hh + 64, :],
                            in0=num_ps[hh][0:DH, :], in1=rb[:], op=Mult,
                        )

            # ================= phase 3: FC =================
            with (
                tc.tile_pool(name="osb", bufs=3) as osbp,
            ):
                for qc in range(8):
                    osb = osbp.tile([128, EMB], f32, tag="ot", name="ot")
                    for cc in range(2):
                        ps = pwork.tile([128, 512], f32, tag="pw", name="pw")
                        for e in range(8):
                            nc.tensor.matmul(
                                ps[:],
                                lhsT=attn_sb[e][:, 128 * qc:128 * qc + 128],
                                rhs=wfc_sb[e][:, 512 * cc:512 * cc + 512],
                                start=(e == 0), stop=(e == 7),
                            )
                        nc.vector.tensor_tensor(
                            out=osb[:, 512 * cc:512 * cc + 512],
                            in0=ps[:],
                            in1=bfc_bc[:, 512 * cc:512 * cc + 512],
                            op=Add,
                        )
                    nc.sync.dma_start(
                        out=out.ap()[128 * qc:128 * qc + 128, :],
                        in_=osb[:])

    nc.compile()
    return nc


def _get_compiled():
    global _compiled
    if _compiled is None:
        _compiled = _build()
    return _compiled


def _make_in_maps(x, w_qkv, b_qkv, w_fc, b_fc):
    wqkv_bf = w_qkv.astype(BF16)
    wfc_bf = w_fc.astype(BF16)
    bqkv_t = np.ascontiguousarray(b_qkv.reshape(24, 128).T.astype(np.float32))
    bv_row = np.ascontiguousarray(b_qkv[2 * EMB:3 * EMB].astype(np.float32))[None, :]
    bfc_row = np.ascontiguousarray(b_fc.astype(np.float32))[None, :]

    tri = np.tril(np.ones((128, 128), dtype=np.float32)).T  # [kv_l, q_l] valid kv<=q
    mask_by_half = {
        0: np.concatenate([tri, np.zeros((128, 128), np.float32)], axis=1),
        1: np.concatenate([np.ones((128, 128), np.float32), tri], axis=1),
    }

    in_maps = []
    for core in range(NCORES):
        b, half = core // 2, core % 2
        xT = x[b].T.astype(BF16)                      # [EMB, S]
        cols = np.concatenate(
            [np.arange(128 * (2 * i + half), 128 * (2 * i + half) + 128)
             for i in range(NSTRIPE)])
        in_maps.append({
            "xT_kv": np.ascontiguousarray(xT),
            "xT_q": np.ascontiguousarray(xT[:, cols]),
            "wqkv": wqkv_bf,
            "wfc": wfc_bf,
            "bqkv_t": bqkv_t,
            "bv_row": bv_row,
            "bfc_row": bfc_row,
            "masks": mask_by_half[half].astype(BF16),
        })
    return in_maps


def kernel(x, w_qkv, b_qkv, w_fc, b_fc, _trace=False):
    from concourse import bass_utils
    from concourse.bass_interp import get_hw_module

    x = np.asarray(x, dtype=np.float32)
    w_qkv = np.asarray(w_qkv, dtype=np.float32)
    b_qkv = np.asarray(b_qkv, dtype=np.float32)
    w_fc = np.asarray(w_fc, dtype=np.float32)
    b_fc = np.asarray(b_fc, dtype=np.float32)

    nc = _get_compiled()
    in_maps = _make_in_maps(x, w_qkv, b_qkv, w_fc, b_fc)

    old_m = nc.m
    nc.m = get_hw_module(nc.m)
    try:
        res = bass_utils.run_bass_kernel_spmd(
            nc, in_maps, core_ids=list(range(NCORES)), trace=_trace)
    finally:
        nc.m = old_m

    y = np.empty((B, S, EMB), dtype=np.float32)
    for core in range(NCORES):
        b, half = core // 2, core % 2
        o = res.results[core]["out"]
        for i in range(NSTRIPE):
            g = 2 * i + half
            y[b, 128 * g:128 * g + 128, :] = o[128 * i:128 * i + 128, :]
    if _trace:
        kernel._last_exec_time_ns = res.exec_time_ns
        kernel._last_results = res
    return y


# revision 12
# speedup vs baseline: 1.1830x; 1.1830x over previous
"""Trainium2 Bass kernel for causal multi-head attention block (nn_Attention).

Reference computation (B=4, S=2048, EMB=1024, H=16, Dh=64):
    qkv = x @ w_qkv + b_qkv ; q,k,v = split(qkv)
    out = softmax(causal(q k^T / sqrt(Dh))) @ v   (per head)
    y = merge_heads(out) @ w_fc + b_fc

Sharding: 8 cores = 4 batches x 2 interleaved query-stripe halves.
Core (b, half) owns query stripes {128*(2i+half) : i=0..7} of batch b.
Each core computes K/V for the full sequence of its batch (zero-collective
SPMD), Q only for its own 1024 queries, causally-tapered attention, and the
final FC for its own output rows. Causal boundary differences between the
two halves are data (mask inputs), not control flow, so all 8 cores run one
graph.

On-device layouts (host pre-transposes x):
    xT_kv [1024,2048] = x[b].T          xT_q [1024,1024] = x[b, stripes].T
    K^T, Q^T computed weight-stationary (lhsT=W chunk, rhs=xT)
    V computed activation-stationary (lhsT=xT chunk, rhs=Wv) -> [s, d] layout
      with a ones-column per head (softmax denominators fall out of PV matmul)
    scores^T [kv,q] = lhsT(K^T).T @ rhs(Q^T); exp on ScalarE (scale=1/8)
    PV: num^T [65, q] = lhsT(V_aug).T @ rhs(probs^T), accumulated over kv tiles
    FC: out [q, c] = lhsT(attn^T).T @ rhs(W_fc)  -> direct row-major output
"""

import numpy as np
import ml_dtypes

B = 4
S = 2048
EMB = 1024
HEADS = 16
DH = 64
NCORES = 8
NSTRIPE = 8          # query stripes per core
QW = NSTRIPE * 128   # queries per core
KV_TILES = 16        # 2048 / 128

BF16 = ml_dtypes.bfloat16

_compiled = None


def _build():
    from concourse import bacc, tile, mybir

    nc = bacc.Bacc("TRN2", target_bir_lowering=False, debug=False,
                   num_devices=NCORES)
    f32 = mybir.dt.float32
    bf16 = mybir.dt.bfloat16
    Exp = mybir.ActivationFunctionType.Exp
    Copy = mybir.ActivationFunctionType.Copy
    Mult = mybir.AluOpType.mult
    Add = mybir.AluOpType.add

    xT_kv = nc.dram_tensor("xT_kv", [EMB, S], bf16, kind="ExternalInput")
    xT_q = nc.dram_tensor("xT_q", [EMB, QW], bf16, kind="ExternalInput")
    wqkv = nc.dram_tensor("wqkv", [EMB, 3 * EMB], bf16, kind="ExternalInput")
    wfc = nc.dram_tensor("wfc", [EMB, EMB], bf16, kind="ExternalInput")
    bqkv_t = nc.dram_tensor("bqkv_t", [128, 24], f32, kind="ExternalInput")
    bv_row = nc.dram_tensor("bv_row", [1, EMB], f32, kind="ExternalInput")
    bfc_row = nc.dram_tensor("bfc_row", [1, EMB], f32, kind="ExternalInput")
    masks = nc.dram_tensor("masks", [128, 256], bf16, kind="ExternalInput")
    out = nc.dram_tensor("out", [QW, EMB], f32, kind="ExternalOutput")

    with tile.TileContext(nc) as tc:
        with (
            tc.tile_pool(name="consts", bufs=1) as consts,
            tc.tile_pool(name="vpool", bufs=1) as vpool,
            tc.tile_pool(name="ktpool", bufs=1) as ktpool,
            tc.tile_pool(name="qtpool", bufs=1) as qtpool,
            tc.tile_pool(name="attn", bufs=1) as attnp,
            tc.tile_pool(name="pwork", bufs=4, space="PSUM") as pwork,
            tc.tile_pool(name="pnum", bufs=2, space="PSUM") as pnum,
        ):
            # ---- constants ----
            mask_sb = consts.tile([128, 256], bf16, tag="mask")
            nc.sync.dma_start(out=mask_sb[:], in_=masks.ap()[:])
            bqkv_sb = consts.tile([128, 24], f32, tag="bqkv")
            nc.sync.dma_start(out=bqkv_sb[:], in_=bqkv_t.ap()[:])
            bv_bc = consts.tile([128, EMB], f32, tag="bvbc")
            bfc_bc = consts.tile([128, EMB], f32, tag="bfcbc")
            with tc.tile_pool(name="brows", bufs=1) as brows:
                bv_sb = brows.tile([1, EMB], f32, tag="bvrow")
                nc.sync.dma_start(out=bv_sb[:], in_=bv_row.ap()[:])
                bfc_sb = brows.tile([1, EMB], f32, tag="bfcrow")
                nc.sync.dma_start(out=bfc_sb[:], in_=bfc_row.ap()[:])
                nc.gpsimd.partition_broadcast(bv_bc[:], bv_sb[:], channels=128)
                nc.gpsimd.partition_broadcast(bfc_bc[:], bfc_sb[:], channels=128)

            # persistent activation tiles
            v_sb = [vpool.tile([128, HEADS, DH + 8], bf16, tag=f"v{st}", name=f"v{st}")
                    for st in range(KV_TILES)]
            kt_sb = [ktpool.tile([128, S], bf16, tag=f"kt{p}", name=f"kt{p}")
                     for p in range(NSTRIPE)]
            qt_sb = [qtpool.tile([128, QW], bf16, tag=f"qt{p}", name=f"qt{p}")
                     for p in range(NSTRIPE)]
            attn_sb = [attnp.tile([128, QW], bf16, tag=f"at{p}", name=f"at{p}")
                       for p in range(NSTRIPE)]

            # ================= phase 1: projections =================
            with (
                tc.tile_pool(name="xkv", bufs=1) as xkvp,
                tc.tile_pool(name="xq", bufs=1) as xqp,
                tc.tile_pool(name="wq", bufs=2) as wqp,
            ):
                xkv = [xkvp.tile([128, S], bf16, tag=f"xk{e}", name=f"xk{e}")
                       for e in range(8)]
                xq = [xqp.tile([128, QW], bf16, tag=f"xq{e}", name=f"xq{e}")
                      for e in range(8)]
                for e in range(8):
                    nc.sync.dma_start(out=xkv[e][:],
                                      in_=xT_kv.ap()[128 * e:128 * e + 128, :])
                    nc.sync.dma_start(out=xq[e][:],
                                      in_=xT_q.ap()[128 * e:128 * e + 128, :])

                # V projection: out[s, d] ; lhsT = xT chunk, rhs = Wv cols
                wv = [wqp.tile([128, EMB], bf16, tag=f"ws{e}", name=f"wv{e}")
                      for e in range(8)]
                for e in range(8):
                    nc.sync.dma_start(
                        out=wv[e][:],
                        in_=wqkv.ap()[128 * e:128 * e + 128, 2048:3072])
                for st in range(KV_TILES):
                    nc.vector.memset(v_sb[st][:, :, DH:DH + 1], 1.0)
                    for dc in range(2):
                        ps = pwork.tile([128, 512], f32, tag="pw", name="pw")
                        for e in range(8):
                            nc.tensor.matmul(
                                ps[:],
                                lhsT=xkv[e][:, 128 * st:128 * st + 128],
                                rhs=wv[e][:, 512 * dc:512 * dc + 512],
                                start=(e == 0), stop=(e == 7),
                            )
                        nc.vector.tensor_tensor(
                            out=v_sb[st][:, 8 * dc:8 * dc + 8, 0:DH],
                            in0=ps[:],
                            in1=bv_bc[:, 512 * dc:512 * dc + 512],
                            op=Add,
                        )

                # K^T projection (pair-tiles of 128 cols = 2 heads)
                wk = [wqp.tile([128, EMB], bf16, tag=f"ws{e}", name=f"wk{e}")
                      for e in range(8)]
                for e in range(8):
                    nc.sync.dma_start(
                        out=wk[e][:],
                        in_=wqkv.ap()[128 * e:128 * e + 128, 1024:2048])
                for p in range(8):
                    for sc in range(4):
                        ps = pwork.tile([128, 512], f32, tag="pw", name="pw")
                        for e in range(8):
                            nc.tensor.matmul(
                                ps[:],
                                lhsT=wk[e][:, 128 * p:128 * p + 128],
                                rhs=xkv[e][:, 512 * sc:512 * sc + 512],
                                start=(e == 0), stop=(e == 7),
                            )
                        nc.vector.tensor_scalar_add(
                            kt_sb[p][:, 512 * sc:512 * sc + 512], ps[:],
                            bqkv_sb[:, 8 + p:9 + p],
                        )

                # Q^T projection (core's own queries only)
                wqt = [wqp.tile([128, EMB], bf16, tag=f"ws{e}", name=f"wqt{e}")
                       for e in range(8)]
                for e in range(8):
                    nc.sync.dma_start(
                        out=wqt[e][:],
                        in_=wqkv.ap()[128 * e:128 * e + 128, 0:1024])
                for p in range(8):
                    for qc in range(2):
                        ps = pwork.tile([128, 512], f32, tag="pw", name="pw")
                        for e in range(8):
                            nc.tensor.matmul(
                                ps[:],
                                lhsT=wqt[e][:, 128 * p:128 * p + 128],
                                rhs=xq[e][:, 512 * qc:512 * qc + 512],
                                start=(e == 0), stop=(e == 7),
                            )
                        nc.vector.tensor_scalar_add(
                            qt_sb[p][:, 512 * qc:512 * qc + 512], ps[:],
                            bqkv_sb[:, p:p + 1],
                        )

            # ================= phase 2: attention =================
            with (
                tc.tile_pool(name="wfc", bufs=1) as wfcp,
                tc.tile_pool(name="osb", bufs=3) as osbp,
            ):
              wfc_sb = [wfcp.tile([128, EMB], bf16, tag=f"wf{e}", name=f"wf{e}")
                        for e in range(8)]
              for e in range(8):
                  nc.sync.dma_start(out=wfc_sb[e][:],
                                    in_=wfc.ap()[128 * e:128 * e + 128, :])
              with (
                tc.tile_pool(name="probs", bufs=6) as probsp,
                tc.tile_pool(name="numsb", bufs=3) as numsbp,
                tc.tile_pool(name="rbp", bufs=3) as rbp,
              ):
                for p in range(8):          # head pair p -> heads 2p, 2p+1
                    num_ps = [pnum.tile([DH + 1, QW], f32, tag="pn", name="pn")
                              for _ in range(2)]
                    for k in range(KV_TILES):
                        q0 = 128 * (k // 2)
                        # absolute q-chunk boundaries, split at PSUM bank edge
                        bounds = [q0, 512, QW] if q0 < 512 else [q0, QW]
                        for hh in range(2):
                            h = 2 * p + hh
                            lo, hi = 64 * hh, 64 * hh + 64
                            probs = probsp.tile([128, QW], bf16, tag="pr", name="pr")
                            for ci in range(len(bounds) - 1):
                                a, bnd = bounds[ci], bounds[ci + 1]
                                n = bnd - a
                                ps = pwork.tile([128, 512], f32, tag="pw", name="pw")
                                nc.tensor.matmul(
                                    ps[:, 0:n],
                                    lhsT=kt_sb[p][lo:hi, 128 * k:128 * k + 128],
                                    rhs=qt_sb[p][lo:hi, a:bnd],
                                    start=True, stop=True,
                                )
                                nc.scalar.activation(
                                    probs[:, a:bnd], ps[:, 0:n], Exp,
                                    scale=0.125,
                                )
                            # mask first 128 q-cols (diag/invalid stripe)
                            nc.vector.tensor_tensor(
                                out=probs[:, q0:q0 + 128],
                                in0=probs[:, q0:q0 + 128],
                                in1=mask_sb[:, 128 * (k % 2):128 * (k % 2) + 128],
                                op=Mult,
                            )
                            # PV accumulate
                            for ci in range(len(bounds) - 1):
                                a, bnd = bounds[ci], bounds[ci + 1]
                                stop = (k == 7 and bnd == 512) or \
                                       (k == 15 and bnd == QW)
                                nc.tensor.matmul(
                                    num_ps[hh][:, a:bnd],
                                    lhsT=v_sb[k][:, h, 0:DH + 1],
                                    rhs=probs[:, a:bnd],
                                    start=(k == 0), stop=stop,
                                )
                    # normalize: attn^T = num^T[0:64] * (1/num^T[64])
                    for hh in range(2):
                        # fused copy+recip of the sums row (PSUM p64 -> SBUF
                        # p64), then down to p0 (partition_broadcast only
                        # reads partition 0 on HW), broadcast, multiply.
                        rr = numsbp.tile([DH + 1, QW], f32, tag="ns", name="ns")
                        nc.vector.tensor_copy(
                            rr[DH:DH + 1, :], num_ps[hh][DH:DH + 1, :])
                        r0 = rbp.tile([1, QW], f32, tag="r0", name="r0")
                        nc.sync.dma_start(out=r0[:], in_=rr[DH:DH + 1, :])
                        nc.vector.reciprocal_approx_fast(out=r0[:], in_=r0[:])
                        rb = rbp.tile([DH, QW], f32, tag="rb", name="rb")
                        nc.gpsimd.partition_broadcast(
                            rb[:], r0[:], channels=DH)
                        nc.vector.tensor_tensor(
                            out=attn_sb[p][64 * hh:64 * hh + 64, :],
                            in0=num_ps[hh][0:DH, :], in1=rb[:], op=Mult,
                        )

              # ================= phase 3: FC =================
              if True:
                for qc in range(8):
                    osb = osbp.tile([128, EMB], f32, tag="ot", name="ot")
                    for cc in range(2):
                        ps = pwork.tile([128, 512], f32, tag="pw", name="pw")
                        for e in range(8):
                            nc.tensor.matmul(
                                ps[:],
                                lhsT=attn_sb[e][:, 128 * qc:128 * qc + 128],
                                rhs=wfc_sb[e][:, 512 * cc:512 * cc + 512],
                                start=(e == 0), stop=(e == 7),
                            )
                        nc.vector.tensor_tensor(
                            out=osb[:, 512 * cc:512 * cc + 512],
                            in0=ps[:],
                            in1=bfc_bc[:, 512 * cc:512 * cc + 512],
                            op=Add,
                        )
                    nc.sync.dma_start(
                        out=out.ap()[128 * qc:128 * qc + 128, :],
                        in_=osb[:])

    nc.compile()
    return nc


def _get_compiled():
    global _compiled
    if _compiled is None:
        _compiled = _build()
    return _compiled


def _make_in_maps(x, w_qkv, b_qkv, w_fc, b_fc):
    wqkv_bf = w_qkv.astype(BF16)
    wfc_bf = w_fc.astype(BF16)
    bqkv_t = np.ascontiguousarray(b_qkv.reshape(24, 128).T.astype(np.float32))
    bv_row = np.ascontiguousarray(b_qkv[2 * EMB:3 * EMB].astype(np.float32))[None, :]
    bfc_row = np.ascontiguousarray(b_fc.astype(np.float32))[None, :]

    tri = np.tril(np.ones((128, 128), dtype=np.float32)).T  # [kv_l, q_l] valid kv<=q
    mask_by_half = {
        0: np.concatenate([tri, np.zeros((128, 128), np.float32)], axis=1),
        1: np.concatenate([np.ones((128, 128), np.float32), tri], axis=1),
    }

    in_maps = []
    for core in range(NCORES):
        b, half = core // 2, core % 2
        xT = x[b].T.astype(BF16)                      # [EMB, S]
        cols = np.concatenate(
            [np.arange(128 * (2 * i + half), 128 * (2 * i + half) + 128)
             for i in range(NSTRIPE)])
        in_maps.append({
            "xT_kv": np.ascontiguousarray(xT),
            "xT_q": np.ascontiguousarray(xT[:, cols]),
            "wqkv": wqkv_bf,
            "wfc": wfc_bf,
            "bqkv_t": bqkv_t,
            "bv_row": bv_row,
            "bfc_row": bfc_row,
            "masks": mask_by_half[half].astype(BF16),
        })
    return in_maps


def kernel(x, w_qkv, b_qkv, w_fc, b_fc, _trace=False):
    from concourse import bass_utils
    from concourse.bass_interp import get_hw_module

    x = np.asarray(x, dtype=np.float32)
    w_qkv = np.asarray(w_qkv, dtype=np.float32)
    b_qkv = np.asarray(b_qkv, dtype=np.float32)
    w_fc = np.asarray(w_fc, dtype=np.float32)
    b_fc = np.asarray(b_fc, dtype=np.float32)

    nc = _get_compiled()
    in_maps = _make_in_maps(x, w_qkv, b_qkv, w_fc, b_fc)

    old_m = nc.m
    nc.m = get_hw_module(nc.m)
    try:
        res = bass_utils.run_bass_kernel_spmd(
            nc, in_maps, core_ids=list(range(NCORES)), trace=_trace)
    finally:
        nc.m = old_m

    y = np.empty((B, S, EMB), dtype=np.float32)
    for core in range(NCORES):
        b, half = core // 2, core % 2
        o = res.results[core]["out"]
        for i in range(NSTRIPE):
            g = 2 * i + half
            y[b, 128 * g:128 * g + 128, :] = o[128 * i:128 * i + 128, :]
    if _trace:
        kernel._last_exec_time_ns = res.exec_time_ns
        kernel._last_results = res
    return y


# revision 13
# speedup vs baseline: 1.1987x; 1.0132x over previous
"""Trainium2 Bass kernel for causal multi-head attention block (nn_Attention).

Reference computation (B=4, S=2048, EMB=1024, H=16, Dh=64):
    qkv = x @ w_qkv + b_qkv ; q,k,v = split(qkv)
    out = softmax(causal(q k^T / sqrt(Dh))) @ v   (per head)
    y = merge_heads(out) @ w_fc + b_fc

Sharding: 8 cores = 4 batches x 2 interleaved query-stripe halves.
Core (b, half) owns query stripes {128*(2i+half) : i=0..7} of batch b.
Each core computes K/V for the full sequence of its batch (zero-collective
SPMD), Q only for its own 1024 queries, causally-tapered attention, and the
final FC for its own output rows. Causal boundary differences between the
two halves are data (mask inputs), not control flow, so all 8 cores run one
graph.

On-device layouts (host pre-transposes x):
    xT_kv [1024,2048] = x[b].T          xT_q [1024,1024] = x[b, stripes].T
    K^T, Q^T computed weight-stationary (lhsT=W chunk, rhs=xT)
    V computed activation-stationary (lhsT=xT chunk, rhs=Wv) -> [s, d] layout
      with a ones-column per head (softmax denominators fall out of PV matmul)
    scores^T [kv,q] = lhsT(K^T).T @ rhs(Q^T); exp on ScalarE (scale=1/8)
    PV: num^T [65, q] = lhsT(V_aug).T @ rhs(probs^T), accumulated over kv tiles
    FC: out [q, c] = lhsT(attn^T).T @ rhs(W_fc)  -> direct row-major output
"""

import numpy as np
import ml_dtypes

B = 4
S = 2048
EMB = 1024
HEADS = 16
DH = 64
NCORES = 8
NSTRIPE = 8          # query stripes per core
QW = NSTRIPE * 128   # queries per core
KV_TILES = 16        # 2048 / 128

BF16 = ml_dtypes.bfloat16

_compiled = None


def _build():
    from concourse import bacc, tile, mybir

    nc = bacc.Bacc("TRN2", target_bir_lowering=False, debug=False,
                   num_devices=NCORES)
    f32 = mybir.dt.float32
    bf16 = mybir.dt.bfloat16
    Exp = mybir.ActivationFunctionType.Exp
    Copy = mybir.ActivationFunctionType.Copy
    Mult = mybir.AluOpType.mult
    Add = mybir.AluOpType.add

    xT_kv = nc.dram_tensor("xT_kv", [EMB, S], bf16, kind="ExternalInput")
    xT_q = nc.dram_tensor("xT_q", [EMB, QW], bf16, kind="ExternalInput")
    wqkv = nc.dram_tensor("wqkv", [EMB, 3 * EMB], bf16, kind="ExternalInput")
    wfc = nc.dram_tensor("wfc", [EMB, EMB], bf16, kind="ExternalInput")
    bqkv_t = nc.dram_tensor("bqkv_t", [128, 24], f32, kind="ExternalInput")
    bv_row = nc.dram_tensor("bv_row", [1, EMB], f32, kind="ExternalInput")
    bfc_row = nc.dram_tensor("bfc_row", [1, EMB], f32, kind="ExternalInput")
    masks = nc.dram_tensor("masks", [128, 256], bf16, kind="ExternalInput")
    out = nc.dram_tensor("out", [QW, EMB], f32, kind="ExternalOutput")

    with tile.TileContext(nc) as tc:
        with (
            tc.tile_pool(name="consts", bufs=1) as consts,
            tc.tile_pool(name="vpool", bufs=1) as vpool,
            tc.tile_pool(name="ktpool", bufs=1) as ktpool,
            tc.tile_pool(name="qtpool", bufs=1) as qtpool,
            tc.tile_pool(name="attn", bufs=1) as attnp,
            tc.tile_pool(name="pwork", bufs=4, space="PSUM") as pwork,
            tc.tile_pool(name="pnum", bufs=2, space="PSUM") as pnum,
        ):
            # ---- constants ----
            mask_sb = consts.tile([128, 256], bf16, tag="mask")
            nc.sync.dma_start(out=mask_sb[:], in_=masks.ap()[:])
            bqkv_sb = consts.tile([128, 24], f32, tag="bqkv")
            nc.sync.dma_start(out=bqkv_sb[:], in_=bqkv_t.ap()[:])
            bv_bc = consts.tile([128, EMB], f32, tag="bvbc")
            bfc_bc = consts.tile([128, EMB], f32, tag="bfcbc")
            with tc.tile_pool(name="brows", bufs=1) as brows:
                bv_sb = brows.tile([1, EMB], f32, tag="bvrow")
                nc.sync.dma_start(out=bv_sb[:], in_=bv_row.ap()[:])
                bfc_sb = brows.tile([1, EMB], f32, tag="bfcrow")
                nc.sync.dma_start(out=bfc_sb[:], in_=bfc_row.ap()[:])
                nc.gpsimd.partition_broadcast(bv_bc[:], bv_sb[:], channels=128)
                nc.gpsimd.partition_broadcast(bfc_bc[:], bfc_sb[:], channels=128)

            # persistent activation tiles
            v_sb = [vpool.tile([128, HEADS, DH + 8], bf16, tag=f"v{st}", name=f"v{st}")
                    for st in range(KV_TILES)]
            kt_sb = [ktpool.tile([128, S], bf16, tag=f"kt{p}", name=f"kt{p}")
                     for p in range(NSTRIPE)]
            qt_sb = [qtpool.tile([128, QW], bf16, tag=f"qt{p}", name=f"qt{p}")
                     for p in range(NSTRIPE)]
            attn_sb = [attnp.tile([128, QW], bf16, tag=f"at{p}", name=f"at{p}")
                       for p in range(NSTRIPE)]

            # ================= phase 1: projections =================
            with (
                tc.tile_pool(name="xkv", bufs=1) as xkvp,
                tc.tile_pool(name="xq", bufs=1) as xqp,
                tc.tile_pool(name="wq", bufs=2) as wqp,
            ):
                xkv = [xkvp.tile([128, S], bf16, tag=f"xk{e}", name=f"xk{e}")
                       for e in range(8)]
                xq = [xqp.tile([128, QW], bf16, tag=f"xq{e}", name=f"xq{e}")
                      for e in range(8)]
                for e in range(8):
                    nc.sync.dma_start(out=xkv[e][:],
                                      in_=xT_kv.ap()[128 * e:128 * e + 128, :])
                    nc.sync.dma_start(out=xq[e][:],
                                      in_=xT_q.ap()[128 * e:128 * e + 128, :])

                # V projection: out[s, d] ; lhsT = xT chunk, rhs = Wv cols
                wv = [wqp.tile([128, EMB], bf16, tag=f"ws{e}", name=f"wv{e}")
                      for e in range(8)]
                for e in range(8):
                    nc.sync.dma_start(
                        out=wv[e][:],
                        in_=wqkv.ap()[128 * e:128 * e + 128, 2048:3072])
                for st in range(KV_TILES):
                    nc.vector.memset(v_sb[st][:, :, DH:DH + 1], 1.0)
                    for dc in range(2):
                        ps = pwork.tile([128, 512], f32, tag="pw", name="pw")
                        for e in range(8):
                            nc.tensor.matmul(
                                ps[:],
                                lhsT=xkv[e][:, 128 * st:128 * st + 128],
                                rhs=wv[e][:, 512 * dc:512 * dc + 512],
                                start=(e == 0), stop=(e == 7),
                            )
                        nc.vector.tensor_tensor(
                            out=v_sb[st][:, 8 * dc:8 * dc + 8, 0:DH],
                            in0=ps[:],
                            in1=bv_bc[:, 512 * dc:512 * dc + 512],
                            op=Add,
                        )

                # K^T projection (pair-tiles of 128 cols = 2 heads)
                wk = [wqp.tile([128, EMB], bf16, tag=f"ws{e}", name=f"wk{e}")
                      for e in range(8)]
                for e in range(8):
                    nc.sync.dma_start(
                        out=wk[e][:],
                        in_=wqkv.ap()[128 * e:128 * e + 128, 1024:2048])
                for p in range(8):
                    for sc in range(4):
                        ps = pwork.tile([128, 512], f32, tag="pw", name="pw")
                        for e in range(8):
                            nc.tensor.matmul(
                                ps[:],
                                lhsT=wk[e][:, 128 * p:128 * p + 128],
                                rhs=xkv[e][:, 512 * sc:512 * sc + 512],
                                start=(e == 0), stop=(e == 7),
                            )
                        nc.vector.tensor_scalar_add(
                            kt_sb[p][:, 512 * sc:512 * sc + 512], ps[:],
                            bqkv_sb[:, 8 + p:9 + p],
                        )

                # Q^T projection (core's own queries only)
                wqt = [wqp.tile([128, EMB], bf16, tag=f"ws{e}", name=f"wqt{e}")
                       for e in range(8)]
                for e in range(8):
                    nc.sync.dma_start(
                        out=wqt[e][:],
                        in_=wqkv.ap()[128 * e:128 * e + 128, 0:1024])
                for p in range(8):
                    for qc in range(2):
                        ps = pwork.tile([128, 512], f32, tag="pw", name="pw")
                        for e in range(8):
                            nc.tensor.matmul(
                                ps[:],
                                lhsT=wqt[e][:, 128 * p:128 * p + 128],
                                rhs=xq[e][:, 512 * qc:512 * qc + 512],
                                start=(e == 0), stop=(e == 7),
                            )
                        nc.vector.tensor_scalar_add(
                            qt_sb[p][:, 512 * qc:512 * qc + 512], ps[:],
                            bqkv_sb[:, p:p + 1],
                        )

            # ================= phase 2: attention =================
            with (
                tc.tile_pool(name="wfc", bufs=1) as wfcp,
                tc.tile_pool(name="osb", bufs=3) as osbp,
            ):
              wfc_sb = [wfcp.tile([128, EMB], bf16, tag=f"wf{e}", name=f"wf{e}")
                        for e in range(8)]
              for e in range(8):
                  nc.sync.dma_start(out=wfc_sb[e][:],
                                    in_=wfc.ap()[128 * e:128 * e + 128, :])
              with (
                tc.tile_pool(name="probs", bufs=6) as probsp,
                tc.tile_pool(name="numsb", bufs=3) as numsbp,
                tc.tile_pool(name="rbp", bufs=3) as rbp,
              ):
                for p in range(8):          # head pair p -> heads 2p, 2p+1
                    num_ps = [pnum.tile([DH + 1, QW], f32, tag="pn", name="pn")
                              for _ in range(2)]
                    for k in range(KV_TILES):
                        q0 = 128 * (k // 2)
                        # absolute q-chunk boundaries, split at PSUM bank edge
                        bounds = [q0, 512, QW] if q0 < 512 else [q0, QW]
                        # scores for BOTH heads first: adjacent K=64 matmuls
                        # in distinct row-groups run concurrently on the PE
                        probs2 = []
                        for hh in range(2):
                            lo, hi = 64 * hh, 64 * hh + 64
                            probs = probsp.tile([128, QW], bf16, tag="pr", name="pr")
                            probs2.append(probs)
                            for ci in range(len(bounds) - 1):
                                a, bnd = bounds[ci], bounds[ci + 1]
                                n = bnd - a
                                ps = pwork.tile([128, 512], f32, tag="pw", name="pw")
                                nc.tensor.matmul(
                                    ps[:, 0:n],
                                    lhsT=kt_sb[p][lo:hi, 128 * k:128 * k + 128],
                                    rhs=qt_sb[p][lo:hi, a:bnd],
                                    start=True, stop=True,
                                )
                                nc.scalar.activation(
                                    probs[:, a:bnd], ps[:, 0:n], Exp,
                                    scale=0.125,
                                )
                            # mask first 128 q-cols (diag/invalid stripe)
                            nc.vector.tensor_tensor(
                                out=probs[:, q0:q0 + 128],
                                in0=probs[:, q0:q0 + 128],
                                in1=mask_sb[:, 128 * (k % 2):128 * (k % 2) + 128],
                                op=Mult,
                            )
                        # PV accumulate for both heads
                        for hh in range(2):
                            h = 2 * p + hh
                            for ci in range(len(bounds) - 1):
                                a, bnd = bounds[ci], bounds[ci + 1]
                                stop = (k == 7 and bnd == 512) or \
                                       (k == 15 and bnd == QW)
                                nc.tensor.matmul(
                                    num_ps[hh][:, a:bnd],
                                    lhsT=v_sb[k][:, h, 0:DH + 1],
                                    rhs=probs2[hh][:, a:bnd],
                                    start=(k == 0), stop=stop,
                                )
                    # normalize: attn^T = num^T[0:64] * (1/num^T[64])
                    for hh in range(2):
                        # fused copy+recip of the sums row (PSUM p64 -> SBUF
                        # p64), then down to p0 (partition_broadcast only
                        # reads partition 0 on HW), broadcast, multiply.
                        rr = numsbp.tile([DH + 1, QW], f32, tag="ns", name="ns")
                        nc.vector.tensor_copy(
                            rr[DH:DH + 1, :], num_ps[hh][DH:DH + 1, :])
                        r0 = rbp.tile([1, QW], f32, tag="r0", name="r0")
                        nc.sync.dma_start(out=r0[:], in_=rr[DH:DH + 1, :])
                        nc.vector.reciprocal_approx_fast(out=r0[:], in_=r0[:])
                        rb = rbp.tile([DH, QW], f32, tag="rb", name="rb")
                        nc.gpsimd.partition_broadcast(
                            rb[:], r0[:], channels=DH)
                        nc.vector.tensor_tensor(
                            out=attn_sb[p][64 * hh:64 * hh + 64, :],
                            in0=num_ps[hh][0:DH, :], in1=rb[:], op=Mult,
                        )

              # ================= phase 3: FC =================
              if True:
                for qc in range(8):
                    osb = osbp.tile([128, EMB], f32, tag="ot", name="ot")
                    for cc in range(2):
                        ps = pwork.tile([128, 512], f32, tag="pw", name="pw")
                        for e in range(8):
                            nc.tensor.matmul(
                                ps[:],
                                lhsT=attn_sb[e][:, 128 * qc:128 * qc + 128],
                                rhs=wfc_sb[e][:, 512 * cc:512 * cc + 512],
                                start=(e == 0), stop=(e == 7),
                            )
                        nc.vector.tensor_tensor(
                            out=osb[:, 512 * cc:512 * cc + 512],
                            in0=ps[:],
                            in1=bfc_bc[:, 512 * cc:512 * cc + 512],
                            op=Add,
                        )
                    nc.sync.dma_start(
                        out=out.ap()[128 * qc:128 * qc + 128, :],
                        in_=osb[:])

    nc.compile()
    return nc


def _get_compiled():
    global _compiled
    if _compiled is None:
        _compiled = _build()
    return _compiled


def _make_in_maps(x, w_qkv, b_qkv, w_fc, b_fc):
    wqkv_bf = w_qkv.astype(BF16)
    wfc_bf = w_fc.astype(BF16)
    bqkv_t = np.ascontiguousarray(b_qkv.reshape(24, 128).T.astype(np.float32))
    bv_row = np.ascontiguousarray(b_qkv[2 * EMB:3 * EMB].astype(np.float32))[None, :]
    bfc_row = np.ascontiguousarray(b_fc.astype(np.float32))[None, :]

    tri = np.tril(np.ones((128, 128), dtype=np.float32)).T  # [kv_l, q_l] valid kv<=q
    mask_by_half = {
        0: np.concatenate([tri, np.zeros((128, 128), np.float32)], axis=1),
        1: np.concatenate([np.ones((128, 128), np.float32), tri], axis=1),
    }

    in_maps = []
    for core in range(NCORES):
        b, half = core // 2, core % 2
        xT = x[b].T.astype(BF16)                      # [EMB, S]
        cols = np.concatenate(
            [np.arange(128 * (2 * i + half), 128 * (2 * i + half) + 128)
             for i in range(NSTRIPE)])
        in_maps.append({
            "xT_kv": np.ascontiguousarray(xT),
            "xT_q": np.ascontiguousarray(xT[:, cols]),
            "wqkv": wqkv_bf,
            "wfc": wfc_bf,
            "bqkv_t": bqkv_t,
            "bv_row": bv_row,
            "bfc_row": bfc_row,
            "masks": mask_by_half[half].astype(BF16),
        })
    return in_maps


def kernel(x, w_qkv, b_qkv, w_fc, b_fc, _trace=False):
    from concourse import bass_utils
    from concourse.bass_interp import get_hw_module

    x = np.asarray(x, dtype=np.float32)
    w_qkv = np.asarray(w_qkv, dtype=np.float32)
    b_qkv = np.asarray(b_qkv, dtype=np.float32)
    w_fc = np.asarray(w_fc, dtype=np.float32)
    b_fc = np.asarray(b_fc, dtype=np.float32)

    nc = _get_compiled()
    in_maps = _make_in_maps(x, w_qkv, b_qkv, w_fc, b_fc)

    old_m = nc.m
    nc.m = get_hw_module(nc.m)
    try:
        res = bass_utils.run_bass_kernel_spmd(
            nc, in_maps, core_ids=list(range(NCORES)), trace=_trace)
    finally:
        nc.m = old_m

    y = np.empty((B, S, EMB), dtype=np.float32)
    for core in range(NCORES):
        b, half = core // 2, core % 2
        o = res.results[core]["out"]
        for i in range(NSTRIPE):
            g = 2 * i + half
            y[b, 128 * g:128 * g + 128, :] = o[128 * i:128 * i + 128, :]
    if _trace:
        kernel._last_exec_time_ns = res.exec_time_ns
        kernel._last_results = res
    return y
